# revision 3
# baseline (speedup 1.0000x reference)
"""Multi-head attention TRN2 kernel, head-sharded across 8 NeuronCores.

Problem: B=4, S=2048, D_IN=512, D_H=512, H=8.
Each core computes one head end-to-end:
    qT/kT = (Wq/Wk).T-projections of x (scores pre-scaled into Wq,bq)
    v     = x @ Wv            (bias bv folded into a host-side constant)
    sT    = kT.T-matmul grid -> scores TRANSPOSED [t, s] so the key mask
            becomes a per-partition bias of the exp() activation
    pT    = exp(sT + maskoff)             (unnormalized probs, transposed)
    oT    = v.T @ pT   and  denom = 1.T @ pT  (extra ones-column matmul)
    part  = (oT.T @ Wp_h) * (1/denom)     (normalization deferred to the end)
Host: out = sum_h part_h + (bp + sum_h bv_h @ Wp_h).

All matmuls run in float32r (1 cycle/row at N=512; ~1.6e-4 rel err).
"""

import math
from contextlib import ExitStack
from functools import lru_cache

import numpy as np

import concourse.bass as bass
import concourse.tile as tile
from concourse import bacc, mybir
from concourse.bass_utils import run_bass_kernel_spmd
from concourse.masks import make_identity

B, S, D, H = 4, 2048, 512, 8
NCORES = 8
MASK_NEG = -30000.0

F32 = mybir.dt.float32
F32R = mybir.dt.float32r
AF = mybir.ActivationFunctionType


def _emit(nc, b_sz, s_sz):
    NT = s_sz // 128      # key tiles (t)
    NSB = s_sz // 512     # query blocks (s)
    NC = D // 128         # 128-chunks of the feature dim

    x_d = nc.dram_tensor("x", [b_sz, s_sz, D], F32, kind="ExternalInput")
    mo_d = nc.dram_tensor("maskoff", [b_sz, 128, NT], F32, kind="ExternalInput")
    wq_d = nc.dram_tensor("wq", [D, D], F32, kind="ExternalInput")
    wk_d = nc.dram_tensor("wk", [D, D], F32, kind="ExternalInput")
    wv_d = nc.dram_tensor("wv", [D, D], F32, kind="ExternalInput")
    wp_d = nc.dram_tensor("wp", [D, D], F32, kind="ExternalInput")
    bq_d = nc.dram_tensor("bq", [128, NC], F32, kind="ExternalInput")
    bk_d = nc.dram_tensor("bk", [128, NC], F32, kind="ExternalInput")
    out_d = nc.dram_tensor("out", [b_sz, s_sz, D], F32, kind="ExternalOutput")

    with tile.TileContext(nc) as tc, ExitStack() as ctx:
        ep = ctx.enter_context
        cpool = ep(tc.tile_pool(name="const", bufs=1))
        wpool = ep(tc.tile_pool(name="w", bufs=1))
        mpool = ep(tc.tile_pool(name="mask", bufs=2))
        xrawp = ep(tc.tile_pool(name="xraw", bufs=3))
        xtp = ep(tc.tile_pool(name="xt", bufs=1))
        ktp = ep(tc.tile_pool(name="kt", bufs=1))
        vp = ep(tc.tile_pool(name="v", bufs=1))
        qtp = ep(tc.tile_pool(name="qt", bufs=2))
        ptp = ep(tc.tile_pool(name="pt", bufs=4))
        otp = ep(tc.tile_pool(name="ot", bufs=2))
        recp = ep(tc.tile_pool(name="rec", bufs=2))
        resp = ep(tc.tile_pool(name="res", bufs=3))
        drp = ep(tc.tile_pool(name="dr", bufs=2, space="DRAM"))
        pop = ep(tc.tile_pool(name="po", bufs=4, space="PSUM"))
        psp = ep(tc.tile_pool(name="pss", bufs=2, space="PSUM"))
        pbp = ep(tc.tile_pool(name="psb", bufs=2, space="PSUM"))

        ident = cpool.tile([128, 128], F32)
        make_identity(nc, ident[:])
        ones_f = cpool.tile([128, 1], F32)
        nc.vector.memset(ones_f[:], 1.0)
        ones = cpool.tile([128, 1], F32R)
        nc.vector.tensor_copy(ones[:], ones_f[:])

        wq = wpool.tile([128, NC, D], F32R)
        wk = wpool.tile([128, NC, D], F32R)
        wv = wpool.tile([128, NC, D], F32R)
        wp = wpool.tile([128, NC, D], F32R)
        for t_, d_ in ((wq, wq_d), (wk, wk_d), (wv, wv_d), (wp, wp_d)):
            nc.sync.dma_start(
                t_[:], d_.ap().rearrange("(c p) e -> p c e", p=128).bitcast(F32R)
            )
        bq_t = wpool.tile([128, NC], F32)
        bk_t = wpool.tile([128, NC], F32)
        nc.sync.dma_start(bq_t[:], bq_d.ap())
        nc.sync.dma_start(bk_t[:], bk_d.ap())

        pending = None  # deferred final-projection stage (oT, rT, b, sb)

        def flush_pending():
            nonlocal pending
            if pending is None:
                return
            oT, rT, b, sb = pending
            for j in range(4):
                pf = pbp.tile([128, 512], F32, tag="pbig")
                for m in range(NC):
                    nc.tensor.matmul(
                        pf[:],
                        oT[:, m, j * 128 : (j + 1) * 128],
                        wp[:, m, :],
                        start=(m == 0),
                        stop=(m == NC - 1),
                    )
                res = resp.tile([128, 512], F32)
                nc.vector.tensor_scalar_mul(res[:], pf[:], rT[:, j : j + 1])
                r0 = sb * 512 + j * 128
                nc.sync.dma_start(out_d.ap()[b, r0 : r0 + 128, :], res[:])
            pending = None

        for b in range(b_sz):
            mo_t = mpool.tile([128, NT], F32)
            nc.sync.dma_start(mo_t[:], mo_d.ap()[b])

            # ---- stage P: transpose x_b, project k^T and v ----
            xT = xtp.tile([128, NC, s_sz], F32R)
            for st in range(NT):
                xr = xrawp.tile([128, D], F32)
                nc.sync.dma_start(xr[:], x_d.ap()[b, st * 128 : (st + 1) * 128, :])
                for c in range(NC):
                    tp = psp.tile([128, 128], F32, tag="psmall")
                    nc.tensor.transpose(tp[:], xr[:, c * 128 : (c + 1) * 128], ident[:])
                    dst = xT[:, c, st * 128 : (st + 1) * 128]
                    if (st * NC + c) % 2 == 0:
                        nc.scalar.activation(dst, tp[:], AF.Copy)
                    else:
                        nc.vector.tensor_copy(dst, tp[:])

            kT = ktp.tile([128, NC, s_sz], F32R)
            v = vp.tile([128, NT, D], F32R)
            for g in range(NT // 4):  # interleave kT (ACT copies) and v (DVE copies)
                for m in range(NC):
                    ps = psp.tile([128, 512], F32, tag="psmall")
                    for c in range(NC):
                        nc.tensor.matmul(
                            ps[:],
                            wk[:, c, m * 128 : (m + 1) * 128],
                            xT[:, c, g * 512 : (g + 1) * 512],
                            start=(c == 0),
                            stop=(c == NC - 1),
                        )
                    nc.scalar.activation(
                        kT[:, m, g * 512 : (g + 1) * 512],
                        ps[:],
                        AF.Identity,
                        bias=bk_t[:, m : m + 1],
                    )
                for tt in range(4):
                    t = g * 4 + tt
                    ps = psp.tile([128, 512], F32, tag="psmall")
                    for c in range(NC):
                        nc.tensor.matmul(
                            ps[:],
                            xT[:, c, t * 128 : (t + 1) * 128],
                            wv[:, c, :],
                            start=(c == 0),
                            stop=(c == NC - 1),
                        )
                    nc.vector.tensor_copy(v[:, t, :], ps[:])

            # ---- stage A: per query-block attention ----
            for sb in range(NSB):
                qT = qtp.tile([128, NC, 512], F32R)
                for m in range(NC):
                    ps = psp.tile([128, 512], F32, tag="psmall")
                    for c in range(NC):
                        nc.tensor.matmul(
                            ps[:],
                            wq[:, c, m * 128 : (m + 1) * 128],
                            xT[:, c, sb * 512 : (sb + 1) * 512],
                            start=(c == 0),
                            stop=(c == NC - 1),
                        )
                    nc.scalar.activation(
                        qT[:, m, :], ps[:], AF.Identity, bias=bq_t[:, m : m + 1]
                    )

                flush_pending()

                po = [
                    pop.tile([128, 512], F32, tag="po", name=f"po{i}")
                    for i in range(NC)
                ]
                pd = pbp.tile([1, 512], F32, tag="pbig")
                for t in range(NT):
                    ps = psp.tile([128, 512], F32, tag="psmall")
                    for c in range(NC):
                        nc.tensor.matmul(
                            ps[:],
                            kT[:, c, t * 128 : (t + 1) * 128],
                            qT[:, c, :],
                            start=(c == 0),
                            stop=(c == NC - 1),
                        )
                    ptile = ptp.tile([128, 512], F32R)
                    nc.scalar.activation(
                        ptile[:], ps[:], AF.Exp, bias=mo_t[:, t : t + 1]
                    )
                    for m in range(NC):
                        nc.tensor.matmul(
                            po[m][:],
                            v[:, t, m * 128 : (m + 1) * 128],
                            ptile[:],
                            start=(t == 0),
                            stop=(t == NT - 1),
                        )
                    nc.tensor.matmul(
                        pd[:], ones[:], ptile[:], start=(t == 0), stop=(t == NT - 1)
                    )

                oT = otp.tile([128, NC, 512], F32R)
                for m in range(NC):
                    nc.vector.tensor_copy(oT[:, m, :], po[m][:])
                den = recp.tile([1, 512], F32)
                nc.vector.tensor_copy(den[:], pd[:])
                dscr = drp.tile([1, 512], F32)
                nc.sync.dma_start(dscr[:], den[:])
                denT = recp.tile([128, 4], F32)
                nc.sync.dma_start(denT[:], dscr[0, :].rearrange("(j p) -> p j", p=128))
                rT = recp.tile([128, 4], F32)
                nc.vector.reciprocal(rT[:], denT[:])

                pending = (oT, rT, b, sb)

        flush_pending()


@lru_cache(maxsize=2)
def _build(b_sz, s_sz):
    nc = bacc.Bacc("TRN2", target_bir_lowering=False, debug=False)
    _emit(nc, b_sz, s_sz)
    nc.compile()
    return nc


def _prep_inputs(x, mask, Wq, bq, Wk, bk, Wv, bv, Wp, bp):
    """Host-side shard prep. Returns (in_maps, bp_eff)."""
    b_sz, s_sz, _ = x.shape
    nt = s_sz // 128
    sc = 1.0 / math.sqrt(D)
    x = np.ascontiguousarray(x, dtype=np.float32)
    # maskoff[b, p, t] = 0 if key (t*128+p) visible else MASK_NEG
    m = np.asarray(mask).reshape(b_sz, s_sz)
    moff = np.where(m != 0, np.float32(0.0), np.float32(MASK_NEG))
    moff = np.ascontiguousarray(moff.reshape(b_sz, nt, 128).transpose(0, 2, 1))

    in_maps = []
    for h in range(NCORES):
        wq_h = np.ascontiguousarray(np.asarray(Wq[h], dtype=np.float32) * sc)
        bq_h = (np.asarray(bq[h], dtype=np.float32) * sc).reshape(4, 128).T
        bk_h = np.asarray(bk[h], dtype=np.float32).reshape(4, 128).T
        in_maps.append(
            {
                "x": x,
                "maskoff": moff,
                "wq": wq_h,
                "wk": np.ascontiguousarray(np.asarray(Wk[h], dtype=np.float32)),
                "wv": np.ascontiguousarray(np.asarray(Wv[h], dtype=np.float32)),
                "wp": np.ascontiguousarray(
                    np.asarray(Wp[h * D : (h + 1) * D, :], dtype=np.float32)
                ),
                "bq": np.ascontiguousarray(bq_h),
                "bk": np.ascontiguousarray(bk_h),
            }
        )
    bv64 = np.asarray(bv, dtype=np.float64)
    wp64 = np.asarray(Wp, dtype=np.float64)
    bp_eff = np.asarray(bp, dtype=np.float64).copy()
    for h in range(NCORES):
        bp_eff += bv64[h] @ wp64[h * D : (h + 1) * D, :]
    return in_maps, bp_eff.astype(np.float32)


def kernel(x, mask, Wq, bq, Wk, bk, Wv, bv, Wp, bp):
    x = np.asarray(x)
    b_sz, s_sz, _ = x.shape
    nc = _build(b_sz, s_sz)
    in_maps, bp_eff = _prep_inputs(x, mask, Wq, bq, Wk, bk, Wv, bv, Wp, bp)
    res = run_bass_kernel_spmd(nc, in_maps, list(range(NCORES)))
    acc = np.zeros((b_sz, s_sz, D), dtype=np.float64)
    for h in range(NCORES):
        acc += res.results[h]["out"]
    acc += bp_eff
    return acc.astype(np.float32)
